# revision 5
# baseline (speedup 1.0000x reference)
"""Trainium2 Bass kernel for nn_AttentionBlock (B=2, T=2048, D=1024, H=16, DH=64).

Strategy: 8-way tensor-parallel over heads (2 heads/core, both batches) for the
attention half; row-sharded FFN (512 rows of the flattened (b,t) axis per core)
with a single 8-rank AllToAll exchanging the attention output; no AllReduce.

All matmuls run as float32r (FP22 multiply, fp32 accumulate, 1 cycle/row for
moving dim >= 256) except the FFN, whose weights are host-cast to bf16.

Self-contained: no imports from the problem directory.
"""

import sys
import types

import numpy as np
import ml_dtypes

import concourse.bass as bass
import concourse.mybir as mybir
import concourse.tile as tile
from concourse import bacc
from concourse.bass_utils import run_bass_kernel_spmd
from concourse.masks import make_identity

N_CORES = 8
P = 128
NEG = -1e9  # additive mask for disallowed logits; exp(NEG) == 0 in fp32
EXP_BIAS = -8.0  # constant subtracted inside exp; cancels in O/l, guards overflow
LN_EPS = 1e-5

F32 = mybir.dt.float32
F32R = mybir.dt.float32r
BF16 = mybir.dt.bfloat16


def _install_profile_shim():
    """bass_utils imports antenv.axon_hooks when trace=True; the module is
    missing from this image. Provide it (and the ctypes-based hook when the
    axon .so is present)."""
    try:
        import antenv
    except ImportError:
        return
    if "antenv.axon_hooks" in sys.modules:
        return
    m = types.ModuleType("antenv.axon_hooks")
    m._hook = None

    def _set(h):
        m._hook = h

    def _get():
        return m._hook

    m.set_axon_ntff_profile_hook = _set
    m.get_axon_ntff_profile_hook = _get
    sys.modules["antenv.axon_hooks"] = m
    antenv.axon_hooks = m
    try:
        from trn_agent_boot.trn_boot import _ntff_profile_via_ctypes

        _set(_ntff_profile_via_ctypes("/opt/axon/libaxon_pjrt.so"))
    except Exception:
        pass


def r32(ap):
    return ap.bitcast(F32R)


def classify_mask(mask, T, XC, YB):
    """Classify the [T,T] bool mask (mask[q,k]) into S^T blocks of
    [YB rows (k), XC cols (q)]. Returns (blocks, bias_tiles):
    blocks[cx] = list of (yb, bias_idx or None); bias_tiles = [n,YB,XC] f32."""
    n_xc, n_yb = T // XC, T // YB
    uniq = {}
    tiles = []
    blocks = []
    for cx in range(n_xc):
        x0 = cx * XC
        lst = []
        for yb in range(n_yb):
            y0 = yb * YB
            sub = mask[x0:x0 + XC, y0:y0 + YB]  # [q, k]
            if not sub.any():
                continue
            if sub.all():
                lst.append((yb, None))
            else:
                bias = np.where(sub.T, np.float32(0), np.float32(NEG))  # [k, q]
                key = bias.tobytes()
                if key not in uniq:
                    uniq[key] = len(tiles)
                    tiles.append(bias)
                lst.append((yb, uniq[key]))
        blocks.append(lst)
    if not tiles:
        tiles.append(np.zeros((YB, XC), np.float32))  # dummy so the input exists
    return blocks, np.stack(tiles).astype(np.float32)


def build(B, T, D, H, blocks, n_bias, ln1_trivial, ln2_trivial, b2_trivial):
    DH = D // H
    HPC = H // N_CORES          # heads per core (2)
    DS = D // P                 # 8 D-subtiles
    NT = T // P                 # 16 t-blocks per batch
    XC = 512                    # q-chunk width
    NX = T // XC                # 4 q-chunks per batch
    ROWS = B * T // N_CORES     # 512 rows per core
    RT = ROWS // P              # 4 row tiles
    DFF = 4 * D
    NHC = DFF // P              # 32 hidden chunks

    nc = bacc.Bacc(trn_type="TRN2", num_devices=N_CORES)

    # ---- DRAM I/O ----
    x_in = nc.dram_tensor("x", [B, T, D], F32, kind="ExternalInput")
    x_rows_in = nc.dram_tensor("x_rows", [ROWS, D], F32, kind="ExternalInput")
    wq_in = nc.dram_tensor("wq", [D, HPC * DH], F32R, kind="ExternalInput")
    wk_in = nc.dram_tensor("wk", [D, HPC * DH], F32R, kind="ExternalInput")
    wv_in = nc.dram_tensor("wv", [D, HPC * DH], F32R, kind="ExternalInput")
    mb_in = nc.dram_tensor("maskbias", [n_bias, P, XC], F32, kind="ExternalInput")
    ln1g_in = nc.dram_tensor("ln1_g", [1, D], F32, kind="ExternalInput")
    ln1b_in = nc.dram_tensor("ln1_b", [1, D], F32, kind="ExternalInput")
    ln2g_in = nc.dram_tensor("ln2_g", [1, D], F32, kind="ExternalInput")
    ln2b_in = nc.dram_tensor("ln2_b", [1, D], F32, kind="ExternalInput")
    w1_in = nc.dram_tensor("w1", [D, DFF], BF16, kind="ExternalInput")
    b1_in = nc.dram_tensor("b1", [DFF], F32, kind="ExternalInput")
    w2_in = nc.dram_tensor("w2", [DFF, D], BF16, kind="ExternalInput")
    b2_in = nc.dram_tensor("b2", [1, D], F32, kind="ExternalInput")
    out = nc.dram_tensor("out", [ROWS, D], F32, kind="ExternalOutput")

    AF = mybir.ActivationFunctionType
    ALU = mybir.AluOpType
    AX = mybir.AxisListType

    with tile.TileContext(nc) as tc:
        with (
            tc.tile_pool(name="cst", bufs=1) as cst,
            tc.tile_pool(name="dram", bufs=1, space="DRAM") as dram,
            tc.tile_pool(name="stat", bufs=8) as stat,
        ):
            # ---------------- constants ----------------
            eps_c = cst.tile([P, 1], F32, tag="eps_c")
            nc.vector.memset(eps_c[:], LN_EPS)
            ebias_c = cst.tile([P, 1], F32, tag="ebias_c")
            nc.vector.memset(ebias_c[:], EXP_BIAS)
            ident = cst.tile([P, P], F32, tag="ident")
            make_identity(nc, ident[:])
            identb = cst.tile([P, P], BF16, tag="identb")
            make_identity(nc, identb[:])

            mbias = []
            for i in range(n_bias):
                t = cst.tile([P, XC], F32, tag=f"mbias{i}", name=f"mbias{i}")
                nc.sync.dma_start(t[:], mb_in[i])
                mbias.append(t)

            b1_sb = cst.tile([P, NHC], F32, tag="b1_sb")
            nc.sync.dma_start(b1_sb[:], b1_in.rearrange("(m p) -> p m", p=P))

            ln2g_bc = ln2b_bc = b2_bc = None
            if not ln2_trivial:
                g_row = cst.tile([1, D], F32, tag="g_row", name="g_row")
                b_row = cst.tile([1, D], F32, tag="b_row", name="b_row")
                nc.sync.dma_start(g_row[:], ln2g_in[:])
                nc.sync.dma_start(b_row[:], ln2b_in[:])
                ln2g_bc = cst.tile([P, D], F32, tag="g_bc", name="g_bc")
                ln2b_bc = cst.tile([P, D], F32, tag="b_bc", name="b_bc")
                nc.gpsimd.partition_broadcast(ln2g_bc[:], g_row[:])
                nc.gpsimd.partition_broadcast(ln2b_bc[:], b_row[:])
            ln1g_bc = ln1b_bc = None
            if not ln1_trivial:
                g1_row = cst.tile([1, D], F32, tag="g1_row", name="g1_row")
                b1_row = cst.tile([1, D], F32, tag="b1_row", name="b1_row")
                nc.sync.dma_start(g1_row[:], ln1g_in[:])
                nc.sync.dma_start(b1_row[:], ln1b_in[:])
                ln1g_bc = cst.tile([P, D], F32, tag="g1_bc", name="g1_bc")
                ln1b_bc = cst.tile([P, D], F32, tag="b1_bc", name="b1_bc")
                nc.gpsimd.partition_broadcast(ln1g_bc[:], g1_row[:])
                nc.gpsimd.partition_broadcast(ln1b_bc[:], b1_row[:])
            if not b2_trivial:
                b2_row = cst.tile([1, D], F32, tag="b2_row", name="b2_row")
                nc.sync.dma_start(b2_row[:], b2_in[:])
                b2_bc = cst.tile([P, D], F32, tag="b2_bc", name="b2_bc")
                nc.gpsimd.partition_broadcast(b2_bc[:], b2_row[:])

            # weights for projections (packed head pairs)
            wq_sb = cst.tile([P, DS, HPC * DH], F32R, tag="wq_sb")
            wk_sb = cst.tile([P, DS, HPC * DH], F32R, tag="wk_sb")
            wv_sb = cst.tile([P, DS, HPC * DH], F32R, tag="wv_sb")
            for wsb, win in ((wq_sb, wq_in), (wk_sb, wk_in), (wv_sb, wv_in)):
                nc.sync.dma_start(wsb[:], win.rearrange("(o p) m -> p o m", p=P))

            # A2A buffers
            a2a_in = dram.tile([B * T // XC * P, XC], F32, tag="a2a_in")
            a2a_out = dram.tile([B * T // XC * P, XC], F32, tag="a2a_out")

            def layer_norm_tile(pool, xt, out_tile, g_bc, b_bc, trivial):
                """LN over free dim of [128, D] tile -> out_tile (fp32).
                Uses out_tile as scratch for the Square pass."""
                sumsq = stat.tile([P, 1], F32, tag="sumsq")
                nmean = stat.tile([P, 1], F32, tag="nmean")
                var = stat.tile([P, 1], F32, tag="var")
                m2 = stat.tile([P, 1], F32, tag="m2")
                istd = stat.tile([P, 1], F32, tag="istd")
                nmi = stat.tile([P, 1], F32, tag="nmi")
                nc.scalar.activation(out_tile[:], xt[:], AF.Square,
                                     accum_out=sumsq[:])
                nc.vector.reduce_sum(nmean[:], xt[:], axis=AX.X, negate=True)
                nc.vector.tensor_scalar_mul(nmean[:], nmean[:], 1.0 / D)
                nc.vector.tensor_tensor(m2[:], nmean[:], nmean[:], ALU.mult)
                nc.vector.tensor_scalar(var[:], sumsq[:], 1.0 / D, None, ALU.mult)
                nc.vector.tensor_tensor(var[:], var[:], m2[:], ALU.subtract)
                nc.scalar.activation(istd[:], var[:], AF.Sqrt, bias=eps_c[:, 0:1])
                nc.vector.reciprocal(istd[:], istd[:])
                nc.vector.tensor_tensor(nmi[:], nmean[:], istd[:], ALU.mult)
                nc.scalar.activation(out_tile[:], xt[:], AF.Identity,
                                     bias=nmi[:], scale=istd[:])
                if not trivial:
                    nc.vector.tensor_tensor(out_tile[:], out_tile[:], g_bc[:], ALU.mult)
                    nc.vector.tensor_tensor(out_tile[:], out_tile[:], b_bc[:], ALU.add)

            # ============ phase 1+2: LN1 + transpose + projections (per batch),
            # ============ then attention; qT/kT/vaug live until attention ends
            with tc.tile_pool(name="qkv", bufs=1) as qkv:
                qT = qkv.tile([P, B, T], F32R, tag="qT")
                kT = qkv.tile([P, B, T], F32R, tag="kT")
                vaug = [
                    qkv.tile([P, NT, DH + 1], F32R, tag=f"vaug{b}_{h}",
                             name=f"vaug{b}_{h}")
                    for b in range(B) for h in range(HPC)
                ]  # index [b*HPC + h]
                for va in vaug:
                    nc.vector.memset(va[:, :, DH:DH + 1].bitcast(F32), 1.0)

                for b in range(B):
                    with (
                        tc.tile_pool(name="ph1", bufs=3) as ph1,
                        tc.tile_pool(name="xnTp", bufs=1) as xnTp,
                        tc.tile_pool(name="tps", bufs=4, space="PSUM") as tps,
                        tc.tile_pool(name="pps", bufs=2, space="PSUM") as pps,
                    ):
                        xnT = xnTp.tile([P, DS, T], F32R, tag="xnT")
                        for tt in range(NT):
                            xt = ph1.tile([P, D], F32, tag="xt")
                            nc.sync.dma_start(
                                xt[:], x_in[b, tt * P:(tt + 1) * P, :])
                            xn = ph1.tile([P, D], F32, tag="xn")
                            layer_norm_tile(ph1, xt, xn, ln1g_bc, ln1b_bc,
                                            ln1_trivial)
                            for ds in range(DS):
                                tp = tps.tile([P, P], F32, tag="tp")
                                nc.tensor.transpose(
                                    tp[:], xn[:, ds * P:(ds + 1) * P], ident[:])
                                nc.vector.tensor_copy(
                                    out=xnT[:, ds, tt * P:(tt + 1) * P], in_=tp[:])

                        # projections: q, k (transposed layout), v via transpose
                        for wsb, dest in ((wq_sb, qT), (wk_sb, kT)):
                            for cxi in range(NX):
                                ps = pps.tile([P, XC], F32, tag="proj_ps")
                                for ds in range(DS):
                                    nc.tensor.matmul(
                                        ps[:], r32(wsb[:, ds, :]),
                                        r32(xnT[:, ds, cxi * XC:(cxi + 1) * XC]),
                                        start=(ds == 0), stop=(ds == DS - 1),
                                    )
                                nc.scalar.activation(
                                    dest[:, b, cxi * XC:(cxi + 1) * XC],
                                    ps[:], AF.Copy)
                        for cxi in range(NX):
                            ps = pps.tile([P, XC], F32, tag="proj_ps")
                            for ds in range(DS):
                                nc.tensor.matmul(
                                    ps[:], r32(wv_sb[:, ds, :]),
                                    r32(xnT[:, ds, cxi * XC:(cxi + 1) * XC]),
                                    start=(ds == 0), stop=(ds == DS - 1),
                                )
                            vt_sb = ph1.tile([P, XC], F32, tag="vt_sb")
                            nc.scalar.activation(vt_sb[:], ps[:], AF.Copy)
                            for tb in range(XC // P):
                                tp = tps.tile([P, P], F32, tag="tp")
                                nc.tensor.transpose(
                                    tp[:], vt_sb[:, tb * P:(tb + 1) * P], ident[:])
                                glob_tb = cxi * (XC // P) + tb
                                for h in range(HPC):
                                    nc.vector.tensor_copy(
                                        out=vaug[b * HPC + h][:, glob_tb, 0:DH],
                                        in_=tp[:, h * DH:(h + 1) * DH],
                                    )

                # =================== attention ===================
                with (
                    tc.tile_pool(name="sps", bufs=3, space="PSUM") as sps,
                    tc.tile_pool(name="opsp", bufs=2, space="PSUM") as opsp,
                    tc.tile_pool(name="psb", bufs=3) as psb,
                    tc.tile_pool(name="nrm", bufs=3) as nrm,
                ):
                    for b in range(B):
                        for h in range(HPC):
                            po = h * DH  # partition offset of head in qT/kT
                            va = vaug[b * HPC + h]
                            for cx in range(NX):
                                blist = blocks[cx]
                                ops = opsp.tile([DH + 1, XC], F32, tag="o_ps")
                                nblk = len(blist)
                                for i, (yb, bidx) in enumerate(blist):
                                    sps_t = sps.tile([P, XC], F32, tag="s_ps")
                                    nc.tensor.matmul(
                                        sps_t[:],
                                        r32(kT[po:po + DH, b, yb * P:(yb + 1) * P]),
                                        r32(qT[po:po + DH, b, cx * XC:(cx + 1) * XC]),
                                        start=True, stop=True,
                                    )
                                    if bidx is not None:
                                        nc.vector.tensor_tensor(
                                            sps_t[:], sps_t[:], mbias[bidx][:],
                                            ALU.add)
                                    pt = psb.tile([P, XC], F32R, tag="p_sb")
                                    nc.scalar.activation(pt[:], sps_t[:], AF.Exp,
                                                         bias=ebias_c[:, 0:1])
                                    nc.tensor.matmul(
                                        ops[:], r32(va[:, yb, :]), r32(pt[:]),
                                        start=(i == 0), stop=(i == nblk - 1),
                                    )
                                # normalize by summed row (ones column of vaug)
                                rl = nrm.tile([1, XC], F32, tag="rl")
                                nc.vector.reciprocal(rl[:], ops[DH:DH + 1, :])
                                rlb = nrm.tile([DH, XC], F32, tag="rlb")
                                nc.gpsimd.partition_broadcast(rlb[:], rl[:])
                                onorm = nrm.tile([DH, XC], F32, tag="onorm")
                                nc.vector.tensor_tensor(
                                    onorm[:], ops[0:DH, :], rlb[:], ALU.mult)
                                shard = b * NX + cx
                                nc.gpsimd.dma_start(
                                    a2a_in[shard * P + po:shard * P + po + DH, :],
                                    onorm[:])

            # =================== all-to-all ===================
            nc.gpsimd.collective_compute(
                "AllToAll", ALU.bypass,
                replica_groups=[list(range(N_CORES))],
                ins=[a2a_in[:]], outs=[a2a_out[:]],
            )

            with tc.tile_pool(name="rows", bufs=1) as rows_pool:
                x_rows = rows_pool.tile([P, RT, D], F32, tag="x_rows")
                nc.sync.dma_start(
                    x_rows[:], x_rows_in.rearrange("(r p) d -> p r d", p=P))
                ln2T = rows_pool.tile([P, DS, ROWS], BF16, tag="ln2T")

                # ======== rows: z = x + attn, LN2, transpose back ========
                with (
                    tc.tile_pool(name="ph4", bufs=3) as ph4,
                    tc.tile_pool(name="attnTp", bufs=1) as attnTp,
                    tc.tile_pool(name="tps2", bufs=2, space="PSUM") as tps2,
                ):
                    attnT = attnTp.tile([P, DS, ROWS], F32, tag="attnT")
                    nc.gpsimd.dma_start(
                        attnT[:], a2a_out.rearrange("(o p) f -> p o f", p=P))
                    for r in range(RT):
                        zt = ph4.tile([P, D], F32, tag="zt")
                        for ds in range(DS):
                            tp = tps2.tile([P, P], F32, tag="tp")
                            nc.tensor.transpose(
                                tp[:], attnT[:, ds, r * P:(r + 1) * P], ident[:])
                            nc.vector.tensor_copy(
                                out=zt[:, ds * P:(ds + 1) * P], in_=tp[:])
                        nc.vector.tensor_tensor(
                            zt[:], zt[:], x_rows[:, r, :], ALU.add)
                        l2 = ph4.tile([P, D], F32, tag="l2")
                        layer_norm_tile(ph4, zt, l2, ln2g_bc, ln2b_bc,
                                        ln2_trivial)
                        l2b = ph4.tile([P, D], BF16, tag="l2b")
                        nc.vector.tensor_copy(out=l2b[:], in_=l2[:])
                        for ds in range(DS):
                            tp2 = tps2.tile([P, P], BF16, tag="tp2")
                            nc.tensor.transpose(
                                tp2[:], l2b[:, ds * P:(ds + 1) * P], identb[:])
                            nc.vector.tensor_copy(
                                out=ln2T[:, ds, r * P:(r + 1) * P], in_=tp2[:])

                # =================== FFN ===================
                with (
                    tc.tile_pool(name="hTp", bufs=1) as hTp,
                    tc.tile_pool(name="w1p", bufs=3) as w1p,
                    tc.tile_pool(name="pps2", bufs=2, space="PSUM") as pps2,
                ):
                    hT = hTp.tile([P, NHC, ROWS], BF16, tag="hT")
                    W1G = 512  # hidden cols per streamed W1 tile
                    for g in range(DFF // W1G):
                        w1t = w1p.tile([P, DS, W1G], BF16, tag="w1t")
                        nc.sync.dma_start(
                            w1t[:],
                            w1_in[:, g * W1G:(g + 1) * W1G]
                            .rearrange("(o p) m -> p o m", p=P))
                        for mi in range(W1G // P):
                            m = g * (W1G // P) + mi
                            hp = pps2.tile([P, ROWS], F32, tag="h_ps")
                            for ds in range(DS):
                                nc.tensor.matmul(
                                    hp[:], w1t[:, ds, mi * P:(mi + 1) * P],
                                    ln2T[:, ds, :],
                                    start=(ds == 0), stop=(ds == DS - 1),
                                )
                            nc.scalar.activation(hT[:, m, :], hp[:], AF.Gelu,
                                                 bias=b1_sb[:, m:m + 1])

                    with (
                        tc.tile_pool(name="w2p", bufs=3) as w2p,
                        tc.tile_pool(name="ops2", bufs=1, space="PSUM") as ops2,
                        tc.tile_pool(name="outp", bufs=1) as outp,
                    ):
                        NDC = D // XC  # 2 output D-chunks
                        KG = 8         # W2 k-subtiles per streamed tile
                        out_sb = [
                            outp.tile([P, D], F32, tag=f"out_sb{r}",
                                      name=f"out_sb{r}")
                            for r in range(RT)
                        ]
                        for n in range(NDC):
                            ops_o = [
                                ops2.tile([P, XC], F32, tag=f"o2_ps{r}",
                                          name=f"o2_ps{r}")
                                for r in range(RT)
                            ]
                            for kg in range(NHC // KG):
                                w2t = w2p.tile([P, KG, XC], BF16, tag="w2t")
                                nc.sync.dma_start(
                                    w2t[:],
                                    w2_in[kg * KG * P:(kg + 1) * KG * P,
                                          n * XC:(n + 1) * XC]
                                    .rearrange("(o p) f -> p o f", p=P))
                                for r in range(RT):
                                    for k in range(KG):
                                        ks = kg * KG + k
                                        nc.tensor.matmul(
                                            ops_o[r][:],
                                            hT[:, ks, r * P:(r + 1) * P],
                                            w2t[:, k, :],
                                            start=(ks == 0),
                                            stop=(ks == NHC - 1),
                                        )
                            for r in range(RT):
                                nc.vector.tensor_tensor(
                                    out_sb[r][:, n * XC:(n + 1) * XC],
                                    ops_o[r][:],
                                    x_rows[:, r, n * XC:(n + 1) * XC], ALU.add)
                                if not b2_trivial:
                                    nc.vector.tensor_tensor(
                                        out_sb[r][:, n * XC:(n + 1) * XC],
                                        out_sb[r][:, n * XC:(n + 1) * XC],
                                        b2_bc[:, n * XC:(n + 1) * XC], ALU.add)
                        for r in range(RT):
                            nc.sync.dma_start(out[r * P:(r + 1) * P, :],
                                              out_sb[r][:])

    nc.finalize()
    return nc


def kernel(x, mask, ln1_g, ln1_b, ln2_g, ln2_b, Wq, Wk, Wv, W1, b1, W2, b2,
           trace=False, trace_kwargs=None):
    _install_profile_shim()
    x = np.asarray(x, dtype=np.float32)
    mask = np.asarray(mask).astype(bool)
    B, T, D = x.shape
    H = Wq.shape[0]
    DH = Wq.shape[2]
    HPC = H // N_CORES
    ROWS = B * T // N_CORES
    XC = 512

    blocks, bias_tiles = classify_mask(mask, T, XC, P)
    ln1_trivial = bool(np.all(ln1_g == 1.0) and np.all(ln1_b == 0.0))
    ln2_trivial = bool(np.all(ln2_g == 1.0) and np.all(ln2_b == 0.0))
    b2_trivial = bool(np.all(b2 == 0.0))

    nc = build(B, T, D, H, blocks, bias_tiles.shape[0],
               ln1_trivial, ln2_trivial, b2_trivial)

    scale = np.float32(1.0 / np.sqrt(DH))
    Wq = np.asarray(Wq, np.float32) * scale
    Wk = np.asarray(Wk, np.float32)
    Wv = np.asarray(Wv, np.float32)
    W1b = np.asarray(W1, np.float32).astype(ml_dtypes.bfloat16)
    W2b = np.asarray(W2, np.float32).astype(ml_dtypes.bfloat16)

    in_maps = []
    for c in range(N_CORES):
        h0 = HPC * c
        r0 = ROWS * c
        bq = r0 // T
        t0 = r0 % T
        m = {
            "x": x,
            "x_rows": np.ascontiguousarray(x[bq, t0:t0 + ROWS, :]),
            "wq": np.ascontiguousarray(
                np.concatenate([Wq[h0 + i] for i in range(HPC)], axis=1)),
            "wk": np.ascontiguousarray(
                np.concatenate([Wk[h0 + i] for i in range(HPC)], axis=1)),
            "wv": np.ascontiguousarray(
                np.concatenate([Wv[h0 + i] for i in range(HPC)], axis=1)),
            "maskbias": bias_tiles,
            "ln1_g": np.asarray(ln1_g, np.float32).reshape(1, D),
            "ln1_b": np.asarray(ln1_b, np.float32).reshape(1, D),
            "ln2_g": np.asarray(ln2_g, np.float32).reshape(1, D),
            "ln2_b": np.asarray(ln2_b, np.float32).reshape(1, D),
            "w1": W1b,
            "b1": np.asarray(b1, np.float32),
            "w2": W2b,
            "b2": np.asarray(b2, np.float32).reshape(1, D),
        }
        in_maps.append(m)

    kw = {}
    if trace:
        kw["trace"] = True
        if trace_kwargs:
            kw.update(trace_kwargs)
    res = run_bass_kernel_spmd(nc, in_maps, core_ids=list(range(N_CORES)), **kw)

    outp = np.empty((B, T, D), np.float32)
    for c in range(N_CORES):
        r0 = ROWS * c
        bq = r0 // T
        t0 = r0 % T
        outp[bq, t0:t0 + ROWS, :] = res.results[c]["out"]
    kernel.last_result = res
    return outp
